# revision 7
# baseline (speedup 1.0000x reference)
"""3-layer GCN (N=50000, E=1.6M + self-loops) on 8 TRN2 NeuronCores.

Strategy (node/data parallel, per sharding hint):
- Nodes sharded by range: core c owns rows [6250c, 6250(c+1)), padded to 6272.
- Per layer: local transform (X@W on own rows, feature-major on PE),
  prescale rows by deg^-1/2, transpose to node-major, AllGather the full
  [50176, 128] bf16 feature table into every core's HBM.
- Aggregation: per 128-dst tile, dma_gather (4 SWDGE queues) pulls the
  src rows for that tile's (dst-sorted, 128-padded) edge slots from the
  table halves (int16 indices), then PE computes aggT = M.T @ S where S
  is the per-chunk one-hot dst-row matrix built on DVE via is_equal
  against a replicated iota. PSUM accumulates across chunks; epilogue
  applies deg^-1/2 postscale + bias + ReLU.
- Layer 3 aggregates first (associativity), then applies W3/b3/ReLU.
Host preprocessing is limited to sharding/index prep (sorting edges,
padding slot runs to a cross-core-uniform layout) and deg^-1/2.
"""
import os

import numpy as np
import ml_dtypes

import concourse.bacc as bacc
import concourse.bass as bass
import concourse.mybir as mybir
import concourse.tile as tile
from concourse.bass_utils import run_bass_kernel_spmd
from concourse.library_config import mlp
from concourse.masks import make_identity

N = 50000
NCORES = 8
PER = 6250
PAD = 6272           # per-core padded node count (49 * 128)
NTILE = PAD // 128   # 49
TROWS = PAD * NCORES  # 50176
HALF = TROWS // 2     # 25088 (= cores 0-3) -- int16-addressable halves
K1 = 12              # 1536 / 128 contraction chunks for layer 1
D0, D1, D2, D3 = 1433, 100, 50, 7
ELEM = 128           # bf16 elements per table row (256B)
CALL_MAX = 1024      # SWDGE ring capacity per dma_gather call
BF16 = ml_dtypes.bfloat16

LAST_EXEC_NS = None
_CACHE = {}


def _ceil128(x):
    return (np.asarray(x) + 127) // 128 * 128


def _wrap_idx(a):
    """[S] int16 -> [128, S//16]; slot i -> (partition i%16 replicated x8, col i//16)."""
    w = a.reshape(-1, 16).T  # [16, S/16]
    return np.tile(w, (8, 1)).astype(np.int16)


def _wrap_slot(a, dtype):
    """[S] -> [128, S//128]; slot i -> (partition i%128, col i//128)."""
    return np.ascontiguousarray(a.reshape(-1, 128).T).astype(dtype)


def _prep_graph(edge_index):
    src = np.asarray(edge_index[0], np.int64)
    dst = np.asarray(edge_index[1], np.int64)
    loops = np.arange(N, dtype=np.int64)
    srcA = np.concatenate([src, loops])
    dstA = np.concatenate([dst, loops])

    deg = np.bincount(dstA, minlength=N).astype(np.float64)
    ds = (1.0 / np.sqrt(deg)).astype(np.float32)  # deg >= 1 via self-loops

    core = dstA // PER
    local = dstA - core * PER
    tilep = local >> 7
    drow = (local & 127).astype(np.int64)
    score = srcA // PER
    trow = score * PAD + (srcA - score * PER)  # table row 0..50175
    half = (trow >= HALF).astype(np.int64)
    idx16 = (trow - half * HALF).astype(np.int64)

    key = ((core * NTILE + tilep) * 2 + half).astype(np.int64)
    order = np.argsort(key, kind="stable")
    key_s = key[order]
    idx_s = idx16[order]
    drow_s = drow[order]

    ngroups = NCORES * NTILE * 2
    counts = np.bincount(key_s, minlength=ngroups).reshape(NCORES, NTILE, 2)
    runlen = _ceil128(counts.max(axis=0))  # [NTILE, 2] common across cores
    runlen = np.maximum(runlen, 128)
    starts = np.zeros(ngroups + 1, np.int64)
    np.cumsum(np.bincount(key_s, minlength=ngroups), out=starts[1:])

    stot = int(runlen.sum())  # slots per core
    idx_pad = np.zeros((NCORES, stot), np.int64)
    drow_pad = np.full((NCORES, stot), 300, np.int64)  # 300 -> all-zero S row
    off = np.zeros(NCORES, np.int64)
    for t in range(NTILE):
        for h in range(2):
            rl = int(runlen[t, h])
            for c in range(NCORES):
                g = (c * NTILE + t) * 2 + h
                n = int(starts[g + 1] - starts[g])
                o = int(off[c])
                idx_pad[c, o : o + n] = idx_s[starts[g] : starts[g + 1]]
                drow_pad[c, o : o + n] = drow_s[starts[g] : starts[g + 1]]
                off[c] = o + rl
    assert (off == stot).all()

    idxw = np.stack([_wrap_idx(idx_pad[c]) for c in range(NCORES)])
    droww = np.stack([_wrap_slot(drow_pad[c].astype(np.float32), BF16) for c in range(NCORES)])
    return ds, runlen, idxw, droww


def _build(runlen, nchunk_max):
    dt = mybir.dt
    stot = int(runlen.sum())
    nchunk_tot = stot // 128

    nc = bacc.Bacc("TRN2", target_bir_lowering=False, debug=False, num_swdge_queues=4)
    xT = nc.dram_tensor("xT", [K1 * 128, PAD], dt.bfloat16, kind="ExternalInput")
    w1 = nc.dram_tensor("w1", [K1, 128, D1], dt.bfloat16, kind="ExternalInput")
    w2 = nc.dram_tensor("w2", [D1, D2], dt.float32, kind="ExternalInput")
    w3 = nc.dram_tensor("w3", [D2, D3], dt.float32, kind="ExternalInput")
    b1 = nc.dram_tensor("b1", [128, 1], dt.float32, kind="ExternalInput")
    b2 = nc.dram_tensor("b2", [128, 1], dt.float32, kind="ExternalInput")
    b3 = nc.dram_tensor("b3", [128, 1], dt.float32, kind="ExternalInput")
    dsrep = nc.dram_tensor("dsrep", [128, PAD], dt.float32, kind="ExternalInput")
    iota = nc.dram_tensor("iota", [128, 128], dt.bfloat16, kind="ExternalInput")
    idxs = nc.dram_tensor("idxs", [128, stot // 16], dt.int16, kind="ExternalInput")
    dstrow = nc.dram_tensor("dstrow", [128, nchunk_tot], dt.bfloat16, kind="ExternalInput")
    out_d = nc.dram_tensor("out", [D3, PAD], dt.float32, kind="ExternalOutput")

    s_spill = nc.dram_tensor("s_spill", [128, nchunk_tot * 128], dt.bfloat16)
    ag_in = [nc.dram_tensor(f"ag_in{i}", [PAD, ELEM], dt.bfloat16) for i in range(3)]
    tables = [
        nc.dram_tensor(f"table{i}", [TROWS, ELEM], dt.bfloat16, addr_space="Shared")
        for i in range(3)
    ]

    # v-groups for transforms / table staging
    groups = [(g * 512, 512) for g in range(PAD // 512)]
    if PAD % 512:
        groups.append((PAD - PAD % 512, PAD % 512))

    with tile.TileContext(nc) as tc:
        with (
            tc.tile_pool(name="const", bufs=1) as constp,
            tc.tile_pool(name="big", bufs=1) as bigp,
            tc.tile_pool(name="slab", bufs=2) as slabp,
            tc.tile_pool(name="stage", bufs=3) as stagep,
            tc.tile_pool(name="m", bufs=2) as mp,
            tc.tile_pool(name="s", bufs=2) as sp,
            tc.tile_pool(name="eptmp", bufs=2) as epp,
            tc.tile_pool(name="psA", bufs=2, space="PSUM") as psA,
            tc.tile_pool(name="psB", bufs=2, space="PSUM") as psB,
            tc.tile_pool(name="psC", bufs=2, space="PSUM") as psC,
        ):
            nc.gpsimd.load_library(mlp)
            # constants
            ident = constp.tile([128, 128], dt.float32)
            make_identity(nc, ident[:])
            w1_sb = constp.tile([128, K1, D1], dt.bfloat16)
            nc.sync.dma_start(
                w1_sb[:],
                bass.AP(w1.ap().tensor, 0, [[D1, 128], [128 * D1, K1], [1, D1]]),
            )
            w2_sb = constp.tile([128, D2], dt.float32)
            nc.sync.dma_start(w2_sb[0:D1, :], w2[:, :])
            w3_sb = constp.tile([128, D3], dt.float32)
            nc.sync.dma_start(w3_sb[0:D2, :], w3[:, :])
            b_sb = []
            for bt in (b1, b2, b3):
                b = constp.tile([128, 1], dt.float32)
                nc.sync.dma_start(b[:], bt[:, :])
                b_sb.append(b)
            dsr = constp.tile([128, PAD], dt.float32)
            nc.sync.dma_start(dsr[:], dsrep[:, :])
            iot = constp.tile([128, 128], dt.bfloat16)
            nc.sync.dma_start(iot[:], iota[:, :])
            idx_sb = constp.tile([128, stot // 16], dt.int16)
            nc.sync.dma_start(idx_sb[:], idxs[:, :])
            drow_sb = constp.tile([128, nchunk_tot], dt.bfloat16)
            nc.sync.dma_start(drow_sb[:], dstrow[:, :])

            out1T = bigp.tile([128, PAD], dt.float32)
            out2T = bigp.tile([128, PAD], dt.float32)
            out3T = bigp.tile([128, PAD], dt.float32)

            qn = [0]

            def stage_table(li, src_big, du, g0, w, src_off=None):
                """prescale src columns [src_off, src_off+w) by ds[g0:g0+w],
                transpose, write ag_in[li] rows [g0, g0+w)."""
                o = g0 if src_off is None else src_off
                ts = epp.tile([128, 512], dt.float32, tag="ts")
                nc.vector.tensor_tensor(
                    ts[0:du, 0:w], src_big[0:du, o : o + w],
                    dsr[0:du, g0 : g0 + w], mybir.AluOpType.mult,
                )
                for s in range(w // 128):
                    pt = psC.tile([128, 128], dt.float32, tag="pt")
                    nc.tensor.transpose(
                        pt[:], ts[0:du, s * 128 : (s + 1) * 128], ident[0:du, :]
                    )
                    st = stagep.tile([128, ELEM], dt.bfloat16, tag="st")
                    nc.vector.tensor_copy(st[:], pt[:])
                    nc.sync.dma_start(
                        ag_in[li][g0 + s * 128 : g0 + (s + 1) * 128, :], st[:]
                    )

            def allgather(li):
                nc.gpsimd.collective_compute(
                    "AllGather",
                    mybir.AluOpType.bypass,
                    replica_groups=[list(range(NCORES))],
                    ins=[ag_in[li].ap().opt()],
                    outs=[tables[li].ap().opt()],
                )

            def aggregate(li, du, epilogue):
                """gather from tables[li] + segsum; epilogue(tile_idx, ps_agg)."""
                table = tables[li]
                halfA = table[0:HALF, :]
                halfB = table[HALF:TROWS, :]
                slot = 0
                chunk = 0
                for t in range(NTILE):
                    nch = int((runlen[t, 0] + runlen[t, 1]) // 128)
                    m = mp.tile([128, nchunk_max, ELEM], dt.bfloat16, tag="m")
                    # gather calls (per half, <=1024 slots each)
                    loff = 0
                    for h in range(2):
                        rl = int(runlen[t, h])
                        src = halfA if h == 0 else halfB
                        ncalls = (rl + CALL_MAX - 1) // CALL_MAX
                        kchunks = rl // 128
                        done = 0
                        for ci in range(ncalls):
                            nik = kchunks // ncalls + (1 if ci < kchunks % ncalls else 0)
                            ni = nik * 128
                            j0 = (loff + done) // 128
                            nc.gpsimd.dma_gather(
                                m[:, j0 : j0 + nik, :],
                                src,
                                idx_sb[:, (slot + done) // 16 : (slot + done + ni) // 16],
                                ni,
                                ni,
                                ELEM,
                                queue_num=qn[0] % 4,
                            )
                            qn[0] += 1
                            done += ni
                        slot += rl
                        loff += rl
                    # one-hot S: build on DVE in layer 0, spill; reload later
                    s_t = sp.tile([128, nchunk_max, 128], dt.bfloat16, tag="s")
                    sd = s_spill[:, chunk * 128 : (chunk + nch) * 128]
                    if li == 0:
                        da = drow_sb[:, chunk : chunk + nch].to_broadcast([128, nch, 128])
                        ia = iot[:, :]
                        ia = bass.AP(ia.tensor, ia.offset, [ia.ap[0], [0, nch], ia.ap[1]])
                        nc.vector.tensor_tensor(
                            s_t[:, 0:nch, :], da, ia, mybir.AluOpType.is_equal
                        )
                        nc.sync.dma_start(sd, s_t[:, 0:nch, :])
                    else:
                        nc.sync.dma_start(s_t[:, 0:nch, :], sd)
                    ps = psA.tile([128, 128], dt.float32, tag="agg")
                    for j in range(nch):
                        nc.tensor.matmul(
                            ps[:],
                            m[:, j, :],
                            s_t[:, j, :],
                            start=(j == 0),
                            stop=(j == nch - 1),
                        )
                    epilogue(t, ps)
                    chunk += nch

            # ---- Layer 1 transform: H1^T = W1^T @ X^T (per v-group) ----
            for g0, w in groups:
                slab = slabp.tile([128, K1, 512], dt.bfloat16, tag="slab")
                nc.sync.dma_start(
                    slab[:, :, 0:w],
                    bass.AP(xT.ap().tensor, g0, [[PAD, 128], [128 * PAD, K1], [1, w]]),
                )
                ph = psB.tile([128, 512], dt.float32, tag="mm")
                for k in range(K1):
                    nc.tensor.matmul(
                        ph[0:D1, 0:w],
                        w1_sb[:, k, :],
                        slab[:, k, 0:w],
                        start=(k == 0),
                        stop=(k == K1 - 1),
                    )
                stage_table(0, ph, D1, g0, w, src_off=0)
            allgather(0)

            def ep1(t, ps):
                sc = epp.tile([128, 128], dt.float32, tag="sc")
                nc.vector.tensor_tensor(
                    sc[0:D1, :], ps[0:D1, :],
                    dsr[0:D1, t * 128 : (t + 1) * 128], mybir.AluOpType.mult,
                )
                nc.scalar.activation(
                    out1T[0:D1, t * 128 : (t + 1) * 128], sc[0:D1, :],
                    mybir.ActivationFunctionType.Relu, bias=b_sb[0][0:D1, :],
                )

            aggregate(0, D1, ep1)

            # ---- Layer 2 transform: H2^T = W2^T @ OUT1^T ----
            for g0, w in groups:
                ph = psB.tile([128, 512], dt.float32, tag="mm")
                nc.tensor.matmul(
                    ph[0:D2, 0:w], w2_sb[0:D1, :], out1T[0:D1, g0 : g0 + w]
                )
                stage_table(1, ph, D2, g0, w, src_off=0)
            allgather(1)

            def ep2(t, ps):
                sc = epp.tile([128, 128], dt.float32, tag="sc")
                nc.vector.tensor_tensor(
                    sc[0:D2, :], ps[0:D2, :],
                    dsr[0:D2, t * 128 : (t + 1) * 128], mybir.AluOpType.mult,
                )
                nc.scalar.activation(
                    out2T[0:D2, t * 128 : (t + 1) * 128], sc[0:D2, :],
                    mybir.ActivationFunctionType.Relu, bias=b_sb[1][0:D2, :],
                )

            aggregate(1, D2, ep2)

            # ---- Layer 3: aggregate first (table3 = ds * OUT2), then W3 ----
            for g0, w in groups:
                stage_table(2, out2T, D2, g0, w)
            allgather(2)

            def ep3(t, ps):
                sc = epp.tile([128, 128], dt.float32, tag="sc")
                nc.vector.tensor_tensor(
                    sc[0:D2, :], ps[0:D2, :],
                    dsr[0:D2, t * 128 : (t + 1) * 128], mybir.AluOpType.mult,
                )
                po = psB.tile([128, 128], dt.float32, tag="mm3")
                nc.tensor.matmul(po[0:D3, :], w3_sb[0:D2, :], sc[0:D2, :])
                nc.scalar.activation(
                    out3T[0:D3, t * 128 : (t + 1) * 128], po[0:D3, :],
                    mybir.ActivationFunctionType.Relu, bias=b_sb[2][0:D3, :],
                )

            aggregate(2, D2, ep3)

            nc.sync.dma_start(out_d[:, :], out3T[0:D3, :])

    nc.compile()
    return nc


def kernel(**inputs):
    global LAST_EXEC_NS
    x = np.asarray(inputs["x"], np.float32)
    ei = np.asarray(inputs["edge_index"])
    W = [np.asarray(inputs[f"W{i}"], np.float32) for i in (1, 2, 3)]
    b = [np.asarray(inputs[f"b{i}"], np.float32) for i in (1, 2, 3)]

    ds, runlen, idxw, droww = _prep_graph(ei)
    nchunk_max = int(((runlen[:, 0] + runlen[:, 1]) // 128).max())
    key = (tuple(runlen.ravel().tolist()), nchunk_max)
    if key not in _CACHE:
        _CACHE[key] = _build(runlen, nchunk_max)
    nc = _CACHE[key]

    # common (replicated) inputs
    w1p = np.zeros((K1 * 128, D1), np.float32)
    w1p[:D0] = W[0]
    w1p = np.ascontiguousarray(w1p.reshape(K1, 128, D1)).astype(BF16)
    bp = []
    for i, d in enumerate((D1, D2, D3)):
        a = np.zeros((128, 1), np.float32)
        a[:d, 0] = b[i]
        bp.append(a)
    iota = np.tile(np.arange(128, dtype=np.float32), (128, 1)).astype(BF16)

    in_maps = []
    for c in range(NCORES):
        sl = slice(c * PER, (c + 1) * PER)
        xTp = np.zeros((K1 * 128, PAD), BF16)
        xTp[:D0, :PER] = x[sl].T.astype(BF16)
        dsl = np.zeros(PAD, np.float32)
        dsl[:PER] = ds[sl]
        in_maps.append(
            {
                "xT": xTp,
                "w1": w1p,
                "w2": W[1],
                "w3": W[2],
                "b1": bp[0],
                "b2": bp[1],
                "b3": bp[2],
                "dsrep": np.ascontiguousarray(np.broadcast_to(dsl, (128, PAD))),
                "iota": iota,
                "idxs": idxw[c],
                "dstrow": droww[c],
            }
        )

    trace = bool(int(os.environ.get("KERNEL_TRACE", "0")))
    if trace:
        import trnprof  # noqa: F401

    res = run_bass_kernel_spmd(nc, in_maps, list(range(NCORES)), trace=trace)
    LAST_EXEC_NS = res.exec_time_ns

    out = np.empty((N, D3), np.float32)
    for c in range(NCORES):
        out[c * PER : (c + 1) * PER] = res.results[c]["out"][:, :PER].T
    return out
